# revision 1
# baseline (speedup 1.0000x reference)
"""BiLinearInteraction Trainium2 kernel (8 NeuronCores, data-parallel over batch).

Reference computation (per pair p=(i,j) of F=26 fields, P=325 pairs):
    out[b, p*64:(p+1)*64] = (x[i, b, :] @ W[p]) * x[j, b, :]
Full shapes: x [26, 4096, 64] f32, W [325, 64, 64] f32 -> out [4096, 20800] f32.

Strategy
- Shard batch axis 4096 -> 8 x 512, replicate W (sharding hint).
- Host pre-packs per-core operands so the device kernel is a pure stream of
  bf16 matmuls + elementwise muls + large contiguous DMAs:
    xn  bf16 [512, 26*64]        natural layout (elementwise xj operand)
    xt  bf16 [64, 4*26*128]      d-major (matmul lhsT), batch-tile-major
    w   bf16 [64, 325*64]        pair-grouped (matmul rhs), pairs sorted by
                                 left field (itertools.combinations order)
- Per batch tile (4 tiles of 128 rows) and left field i (pairs (i, i+1..25)
  are contiguous): matmul psum[128, n_i*64] = xt_i.T @ w[:, pair cols], then
  one DVE mul with xn[:, (i+1)*64:(i+1+n_i)*64] (right fields j are
  consecutive) into an SBUF staging chunk; chunks of whole fields are DMAed
  out as large contiguous transfers on the SP HWDGE ring while input loads
  ride SWDGE, keeping the write stream (the critical path: 42.6MB/core at
  ~358GB/s) unblocked. Measured ~135.6us on HW, ~= the HBM roofline for
  48.7MB/core of traffic.
"""

import sys

sys.path.insert(0, "/opt/trn_rl_repo")

from itertools import combinations

import ml_dtypes
import numpy as np

import concourse.bass as bass
import concourse.mybir as mybir
from concourse import bacc
from concourse.tile import TileContext

F, D, B = 26, 64, 4096
NCORES = 8
BC = B // NCORES          # 512 batch rows per core
NT = BC // 128            # 4 batch tiles of 128 rows
PAIRS = list(combinations(range(F), 2))
P = len(PAIRS)            # 325
OUT_COLS = P * D          # 20800

# Fields grouped into output chunks (pair counts 25,24,...,1). Whole-field
# chunks keep both the staging tile and the xj slice contiguous. The leading
# chunks are single fields so the first output write starts as early as
# possible — the SP-ring write stream is the kernel's critical path.
CHUNKS = [range(0, 2), range(2, 4), range(4, 6), range(6, 10),
          range(10, 14), range(14, 19), range(19, 25)]
N_PAIRS = [F - 1 - i for i in range(F - 1)]          # pairs with left field i
P_START = [sum(N_PAIRS[:i]) for i in range(F - 1)]   # first pair index of field i

F32 = mybir.dt.float32
BF16 = mybir.dt.bfloat16


def build_bass() -> bass.Bass:
    # Bacc (not Bass): its compile() splits multi-sem waits into event
    # semaphores — TRN2 engine instructions take at most one inline wait.
    nc = bacc.Bacc()
    xn = nc.declare_dram_parameter("xn", [BC, F * D], BF16, isOutput=False)
    # xt/w carry two stacked copies (partitions 0-63 and 64-127): paired
    # fields run as concurrent matmuls in the two 64-row groups of the
    # PE array (tile_position row tiling), halving effective PE time.
    xt = nc.declare_dram_parameter("xt", [2 * D, NT * F * 128], BF16, isOutput=False)
    w = nc.declare_dram_parameter("w", [2 * D, OUT_COLS], BF16, isOutput=False)
    # Output staged and written as bf16 (halves the 42.6MB/core write
    # stream, the kernel's critical path); host upcasts to f32.
    out = nc.declare_dram_parameter("out", [BC, OUT_COLS], BF16, isOutput=True)

    with TileContext(nc) as tc:
        with (
            tc.tile_pool(name="consts", bufs=1) as consts,
            tc.tile_pool(name="xn_pool", bufs=2) as xn_pool,
            tc.tile_pool(name="stage", bufs=5) as stage_pool,
            tc.tile_pool(name="cp_pool", bufs=3) as cp_pool,
            tc.tile_pool(name="psum", bufs=2, space="PSUM") as psum_pool,
        ):
            # Separate tiles per weight chunk / per xt batch-tile: dependency
            # granularity stays per-piece, and the just-in-time emission order
            # below means the first chunk's operands land ~10us before the
            # rest instead of gating the first matmul on all 4.4MB.
            cbounds = []
            for ch in CHUNKS:
                c0 = P_START[ch[0]] * D
                c1 = (P_START[ch[-1]] + N_PAIRS[ch[-1]]) * D
                cbounds.append((c0, c1))
            w_sb = [consts.tile([2 * D, c1 - c0], BF16, tag=f"w{ci}", name=f"w{ci}")
                    for ci, (c0, c1) in enumerate(cbounds)]
            xt_sb = [consts.tile([2 * D, F * 128], BF16, tag=f"xt{t}", name=f"xtsb{t}")
                     for t in range(NT)]

            # Input loads go through SWDGE (nc.gpsimd) — a separate DMA
            # descriptor path from the SP HWDGE ring carrying the output
            # writes. HWDGE is strict FIFO per ring: with everything on one
            # ring, every t>0 input load queues behind ~10MB of output
            # writes (measured ~15us pipeline stalls per batch-tile
            # boundary).
            c0, c1 = cbounds[0]
            nc.gpsimd.dma_start(out=w_sb[0][:], in_=w[:, c0:c1])
            nc.gpsimd.dma_start(out=xt_sb[0][:], in_=xt[:, 0:F * 128])
            xn_tiles = []
            xn_sb = xn_pool.tile([128, F * D], BF16, tag="xn")
            nc.gpsimd.dma_start(out=xn_sb[:], in_=xn[0:128, :])
            xn_tiles.append(xn_sb)

            for t in range(NT):
                if t > 0:
                    s = t * F * 128
                    nc.gpsimd.dma_start(out=xt_sb[t][:], in_=xt[:, s:s + F * 128])
                    xn_sb = xn_pool.tile([128, F * D], BF16, tag="xn")
                    nc.gpsimd.dma_start(
                        out=xn_sb[:], in_=xn[t * 128:(t + 1) * 128, :]
                    )
                else:
                    xn_sb = xn_tiles[0]
                for ci, ch in enumerate(CHUNKS):
                    if t == 0 and ci + 1 < len(CHUNKS):
                        nc0, nc1 = cbounds[ci + 1]
                        nc.gpsimd.dma_start(out=w_sb[ci + 1][:], in_=w[:, nc0:nc1])
                    ccol0, ccol1 = cbounds[ci]
                    ccols = ccol1 - ccol0
                    st = stage_pool.tile([128, ccols], BF16, tag="stage")
                    for i in ch:
                        npair = N_PAIRS[i]
                        cols = npair * D
                        wcol0 = P_START[i] * D
                        ps = psum_pool.tile([128, cols], F32, tag="ps")
                        r0 = (i % 2) * D  # PE row group alternates per field
                        lhsT = xt_sb[t][r0:r0 + D, i * 128:(i + 1) * 128]
                        for s0 in range(0, cols, 512):
                            n = min(512, cols - s0)
                            nc.tensor.matmul(
                                ps[:, s0:s0 + n], lhsT,
                                w_sb[ci][r0:r0 + D,
                                         wcol0 - ccol0 + s0:wcol0 - ccol0 + s0 + n],
                                start=True, stop=True,
                            )
                        if i < 8:
                            # Large fields: ScalarE drains PSUM (f32->bf16);
                            # the DVE mul then runs all-bf16/all-SBUF, which
                            # qualifies for the packed 2x DVE mode. Splits the
                            # PSUM-drain work across the otherwise-idle ACT.
                            cp = cp_pool.tile([128, cols], BF16, tag="cp")
                            nc.scalar.copy(out=cp[:], in_=ps[:])
                            nc.vector.tensor_mul(
                                st[:, wcol0 - ccol0:wcol0 - ccol0 + cols],
                                cp[:],
                                xn_sb[:, (i + 1) * D:(i + 1 + npair) * D],
                            )
                        else:
                            nc.vector.tensor_mul(
                                st[:, wcol0 - ccol0:wcol0 - ccol0 + cols],
                                ps[:],
                                xn_sb[:, (i + 1) * D:(i + 1 + npair) * D],
                            )
                    nc.sync.dma_start(
                        out=out[t * 128:(t + 1) * 128, ccol0:ccol0 + ccols],
                        in_=st[:],
                    )
    nc.compile()
    return nc


def prep_inputs(x: np.ndarray, W: np.ndarray):
    """Full inputs -> per-core in_maps with pre-packed layouts."""
    x = np.ascontiguousarray(np.asarray(x, dtype=np.float32))
    W = np.ascontiguousarray(np.asarray(W, dtype=np.float32))
    # w: [D, P*D], col = p*64 + e, bf16; identical on every core.
    wg = W.transpose(1, 0, 2).reshape(D, OUT_COLS).astype(ml_dtypes.bfloat16)
    wg = np.ascontiguousarray(np.concatenate([wg, wg], axis=0))  # both row groups
    in_maps = []
    for c in range(NCORES):
        xc = x[:, c * BC:(c + 1) * BC, :]                      # [26, 512, 64]
        xn = np.ascontiguousarray(
            xc.transpose(1, 0, 2).reshape(BC, F * D).astype(ml_dtypes.bfloat16)
        )
        xt1 = (xc.reshape(F, NT, 128, D).transpose(3, 1, 0, 2)
               .reshape(D, NT * F * 128).astype(ml_dtypes.bfloat16))
        xt = np.ascontiguousarray(np.concatenate([xt1, xt1], axis=0))
        in_maps.append({"xn": xn, "xt": xt, "w": wg})
    return in_maps


_CACHED_NC = None


def kernel(x: np.ndarray, W: np.ndarray) -> np.ndarray:
    global _CACHED_NC
    from concourse.bass_utils import run_bass_kernel_spmd

    if _CACHED_NC is None:
        _CACHED_NC = build_bass()
    in_maps = prep_inputs(x, W)
    res = run_bass_kernel_spmd(_CACHED_NC, in_maps, list(range(NCORES)))
    shards = [
        np.asarray(res.results[c]["out"]).astype(np.float32) for c in range(NCORES)
    ]
    return np.concatenate(shards, axis=0)



# revision 31
# speedup vs baseline: 1.0763x; 1.0763x over previous
"""BiLinearInteraction Trainium2 kernel (8 NeuronCores, data-parallel over batch).

Reference computation (per pair p=(i,j) of F=26 fields, P=325 pairs):
    out[b, p*64:(p+1)*64] = (x[i, b, :] @ W[p]) * x[j, b, :]
Full shapes: x [26, 4096, 64] f32, W [325, 64, 64] f32 -> out [4096, 20800] f32.

Strategy (v2)
- Shard batch 4096 -> 8 x 512 (4 batch tiles of 128 rows per core), replicate W.
- Parity-packed operands: even fields' matmul data on SBUF partitions 0-63,
  odd fields' on 64-127 (PE row groups run concurrently via tile_position),
  with NO duplication -> 6.1 MB/core of input HBM traffic (was 10.4).
- All input loads ride the ACT HWDGE ring, output writes the SP HWDGE ring;
  GPSIMD does no DMA descriptor work and is free to run elementwise muls.
- Per batch tile, matmuls accumulate into [128, <=2048] PSUM group tiles
  (4 banks, 2 bufs = whole PSUM). Groups are classed V/A/G:
    V: DVE multiplies straight out of PSUM (fp32, 1x) into the bf16 stage.
    A: one big ACT copy drains the group PSUM -> stage (bf16), then DVE
       multiplies in place at 2x (all-bf16 packed mode).
    G: same drain, but GPSIMD does the in-place mul.
  This spreads the 10.65M-elem/core evacuate+multiply load across all three
  engines (~60us each) instead of DVE+ACT only (~95us DVE in v1).
- Output staged per half batch-tile and written as two ~2.5MB DMAs per tile
  (bf16; host upcasts) -> ~385 GB/s ring efficiency vs ~340 at 0.8MB chunks.
"""

import os
import sys

sys.path.insert(0, "/opt/trn_rl_repo")

from itertools import combinations

import ml_dtypes
import numpy as np

import concourse.bass as bass
import concourse.mybir as mybir
from concourse import bacc
from concourse.tile import TileContext

F, D, B = 26, 64, 4096
NCORES = 8
BC = B // NCORES          # 512 batch rows per core
NT = BC // 128            # 4 batch tiles of 128 rows
PAIRS = list(combinations(range(F), 2))
P = len(PAIRS)            # 325
OUT_COLS = P * D          # 20800

N_PAIRS = [F - 1 - i for i in range(F - 1)]           # pairs with left field i
P_START = [sum(N_PAIRS[:i]) for i in range(F - 1)]    # first pair index of field i
FIELD_START = [P_START[i] * D for i in range(F - 1)]  # output col where field i begins
FIELD_END = [FIELD_START[i] + N_PAIRS[i] * D for i in range(F - 1)]

XTW = F * 128             # xt cols per batch tile (all fields, both halves) = 3328
XNW = F * D               # xn cols per batch tile                           = 1664

# PSUM group grid per batch tile: (c0, c1, class).  <=PSUM_G f32 cols/group.
#   V: DVE mul direct from PSUM; A: ACT drain + DVE in-place mul;
#   G: ACT drain + GPSIMD in-place mul.
# Halves are field-aligned (field 7 starts at 9856) so muls never span them.
# Contiguous class runs (mul granularity for A/G: field pieces within a run).
RUNS = [(0, 6144, 'V'), (6144, 9856, 'A'), (9856, 16000, 'G'),
        (16000, 20800, 'A')]
HALF = 9856               # st0 covers cols [0, 9856), st1 [9856, 20800)
PSUM_G = int(os.environ.get("K_PSUM_G", "2048"))


def _make_groups():
    groups = []
    for (r0, r1, cls) in RUNS:
        # split the run at the half boundary first, then into <=PSUM_G chunks
        segs = []
        if r0 < HALF < r1:
            segs = [(r0, HALF), (HALF, r1)]
        else:
            segs = [(r0, r1)]
        for (s0, s1) in segs:
            n = -(-(s1 - s0) // PSUM_G)
            step = -(-(s1 - s0) // n)
            step = -(-step // 64) * 64   # keep 64-col alignment
            c = s0
            while c < s1:
                groups.append((c, min(c + step, s1), cls))
                c += step
    return groups


GROUPS = _make_groups()

# PSUM 512-col blocks (bank-aligned within each group's psum tile).  The PE
# row group of every matmul is the BLOCK index parity, not the field parity:
# two concurrent (different-row-group) matmuls writing the same PSUM bank is
# a fatal HW collision, so same-bank pieces must share a row group, while
# alternating adjacent blocks keeps dual-row-group concurrency.
# W is packed by block parity: top half (partitions 0-63) holds even blocks'
# columns, bottom half odd blocks'.  xt carries all fields in BOTH halves.
BLOCKS = []               # (c0, c1, parity, w_off)
_tops = _bots = 0
_idx = 0
_V_END_OFF = 0
for (_g0, _g1, _cls) in GROUPS:
    _c = _g0
    while _c < _g1:
        _c1 = min(_c + 512, _g1)
        _par = _idx % 2
        if _par == 0:
            _off = _tops
            _tops += _c1 - _c
        else:
            _off = _bots
            _bots += _c1 - _c
        BLOCKS.append((_c, _c1, _par, _off))
        _idx += 1
        _c = _c1
    if _cls == 'V' and _g1 == RUNS[0][1]:
        _V_END_OFF = max(_tops, _bots)
W_COLS = max(_tops, _bots)
W_SPLIT = -(-_V_END_OFF // 512) * 512     # block-aligned in both halves


def _block_of(col):
    for b in BLOCKS:
        if b[0] <= col < b[1]:
            return b
    raise ValueError(col)

F32 = mybir.dt.float32
BF16 = mybir.dt.bfloat16


def _even_splits(c0, c1, n):
    step = -(-((c1 - c0) // n) // 64) * 64
    step = max(step, 64)
    out = []
    c = c0
    while c < c1:
        out.append((c, min(c + step, c1)))
        c += step
    return out


def _field_of(col):
    for i in range(F - 1):
        if FIELD_START[i] <= col < FIELD_END[i]:
            return i
    raise ValueError(col)


def _pieces(c0, c1, extra=()):
    """Split [c0, c1) at field starts and any extra boundaries.
    Returns list of (p0, p1, field)."""
    bounds = {c0, c1}
    bounds.update(s for s in FIELD_START if c0 < s < c1)
    bounds.update(e for e in extra if c0 < e < c1)
    bs = sorted(bounds)
    return [(a, b, _field_of(a)) for a, b in zip(bs, bs[1:])]


def _mm_pieces(g0, g1):
    """Matmul pieces: additionally split at 512-col PSUM bank boundaries
    (relative to the group base = block boundaries)."""
    extra = set(range(g0 + 512, g1, 512))
    return _pieces(g0, g1, extra)


def build_bass() -> bass.Bass:
    # Bisection flags (default off = full-featured kernel).
    no_gps = os.environ.get("K_NO_GPS", "0") == "1"       # gpsimd muls -> DVE
    no_inplace = os.environ.get("K_NO_INPLACE", "0") == "1"  # muls via cp tile
    swdge_loads = os.environ.get("K_SWDGE_LOADS", "0") == "1"  # loads on gpsimd
    wsplit = int(os.environ.get("K_WRITE_SPLIT", "1"))    # write DMAs per half
    nt_limit = int(os.environ.get("K_NT_LIMIT", str(NT)))  # batch tiles to run
    ngroups = int(os.environ.get("K_NGROUPS", str(len(GROUPS))))
    no_muls = os.environ.get("K_NO_MULS", "0") == "1"
    no_drains = os.environ.get("K_NO_DRAINS", "0") == "1"
    nc = bacc.Bacc()
    xn = nc.declare_dram_parameter("xn", [128, NT * XNW], BF16, isOutput=False)
    xt = nc.declare_dram_parameter("xt", [128, NT * XTW], BF16, isOutput=False)
    w = nc.declare_dram_parameter("w", [128, W_COLS], BF16, isOutput=False)
    out = nc.declare_dram_parameter("out", [BC, OUT_COLS], BF16, isOutput=True)

    with TileContext(nc) as tc:
        with (
            tc.tile_pool(name="consts", bufs=1) as consts,
            tc.tile_pool(name="stage", bufs=3) as stage_pool,
            tc.tile_pool(name="cp", bufs=2) as cp_pool,
            tc.tile_pool(name="psum", bufs=2, space="PSUM") as psum_pool,
        ):
            # Separate tile objects per load DMA keep dependency granularity
            # at the piece level even if subtile range tracking is coarse.
            w_a = consts.tile([128, W_SPLIT], BF16, tag="wa", name="wa")
            w_b = consts.tile([128, W_COLS - W_SPLIT], BF16, tag="wb", name="wb")
            xt0 = consts.tile([128, XTW], BF16, tag="xt0", name="xt0")
            xtr = consts.tile([128, (NT - 1) * XTW], BF16, tag="xtr", name="xtr")
            xn0 = consts.tile([128, XNW], BF16, tag="xn0", name="xn0")
            xnr = consts.tile([128, (NT - 1) * XNW], BF16, tag="xnr", name="xnr")

            # Input loads on the ACT HWDGE ring, ordered so the first batch
            # tile's operands land first (first matmul ~5us in).
            ldeng = nc.gpsimd if swdge_loads else nc.scalar
            ldeng.dma_start(out=xt0[:], in_=xt[:, 0:XTW])
            ldeng.dma_start(out=w_a[:], in_=w[:, 0:W_SPLIT])
            ldeng.dma_start(out=xn0[:], in_=xn[:, 0:XNW])
            ldeng.dma_start(out=w_b[:], in_=w[:, W_SPLIT:W_COLS])
            ldeng.dma_start(out=xtr[:], in_=xt[:, XTW:NT * XTW])
            ldeng.dma_start(out=xnr[:], in_=xn[:, XNW:NT * XNW])

            def xt_slice(t, i, r0):
                c = i * 128
                if t == 0:
                    return xt0[r0:r0 + D, c:c + 128]
                c += (t - 1) * XTW
                return xtr[r0:r0 + D, c:c + 128]

            def w_slice(c0, c1):
                b0, b1, par, boff = _block_of(c0)
                assert c1 <= b1, (c0, c1, b0, b1)
                r0 = par * D
                wc = boff + (c0 - b0)
                n = c1 - c0
                if wc < W_SPLIT:
                    assert wc + n <= W_SPLIT, (c0, c1)
                    return r0, w_a[r0:r0 + D, wc:wc + n]
                return r0, w_b[r0:r0 + D, wc - W_SPLIT:wc - W_SPLIT + n]

            def xn_slice(t, i, c0, c1):
                c = (i + 1) * D + (c0 - FIELD_START[i])
                if t > 0:
                    c += (t - 1) * XNW
                src = xn0 if t == 0 else xnr
                return src[:, c:c + (c1 - c0)]

            for t in range(nt_limit):
                st0 = stage_pool.tile([128, HALF], BF16, tag="stage",
                                      name=f"st{t}a")
                st1 = stage_pool.tile([128, OUT_COLS - HALF], BF16, tag="stage",
                                      name=f"st{t}b")

                def st_slice(c0, c1):
                    if c0 >= HALF:
                        return st1[:, c0 - HALF:c1 - HALF]
                    assert c1 <= HALF
                    return st0[:, c0:c1]

                if no_muls:
                    nc.vector.memset(st0[:], 0.0)
                    nc.vector.memset(st1[:], 0.0)

                # A/G-run mul pieces not yet emitted, per run index.
                pending = {}
                cp_tiles = {}
                for ri, (r0_, r1_, rcls) in enumerate(RUNS):
                    if rcls != 'V':
                        pending[ri] = _pieces(r0_, r1_)
                        if no_inplace:
                            cp_tiles[ri] = cp_pool.tile(
                                [128, r1_ - r0_], BF16, tag="cp",
                                name=f"cp{t}_{ri}")

                gskip = int(os.environ.get("K_GSKIP", "0"))
                mm_filter = os.environ.get("K_MM_FILTER")
                if mm_filter is not None:
                    mm_filter = {int(v) for v in mm_filter.split(",")}
                mm_idx = 0
                for (g0, g1, gcls) in GROUPS[gskip:ngroups]:
                    ps = psum_pool.tile([128, g1 - g0], F32, tag="ps",
                                        name=f"ps{t}_{g0}")
                    for (c0, c1, i) in _mm_pieces(g0, g1):
                        mm_idx += 1
                        if mm_filter is not None and (mm_idx - 1) not in mm_filter:
                            continue
                        r0, rhs = w_slice(c0, c1)
                        nc.tensor.matmul(
                            ps[:, c0 - g0:c1 - g0],
                            xt_slice(t, i, r0),
                            rhs,
                            start=True, stop=True,
                        )
                    if gcls == 'V':
                        for (c0, c1, i) in _pieces(g0, g1):
                            if no_muls:
                                break
                            nc.vector.tensor_mul(
                                st_slice(c0, c1),
                                ps[:, c0 - g0:c1 - g0],
                                xn_slice(t, i, c0, c1),
                            )
                    else:
                        ri = next(k for k, (a, b, cl) in enumerate(RUNS)
                                  if a <= g0 < b)
                        run0 = RUNS[ri][0]
                        # One big ACT drain (f32 PSUM -> bf16, cast).
                        if no_inplace:
                            drain_dst = cp_tiles[ri][:, g0 - run0:g1 - run0]
                        else:
                            drain_dst = st_slice(g0, g1)
                        if not no_drains:
                            nc.scalar.copy(out=drain_dst, in_=ps[:])
                        # Emit muls for run pieces fully drained now.
                        eng = nc.vector if (gcls == 'A' or no_gps) else nc.gpsimd
                        done = [pc for pc in pending[ri] if pc[1] <= g1]
                        for (c0, c1, i) in done:
                            pending[ri].remove((c0, c1, i))
                            if no_muls:
                                continue
                            if no_inplace:
                                msrc = cp_tiles[ri][:, c0 - run0:c1 - run0]
                            else:
                                msrc = st_slice(c0, c1)
                            eng.tensor_mul(
                                st_slice(c0, c1),
                                msrc,
                                xn_slice(t, i, c0, c1),
                            )
                    if g1 == HALF:
                        for (a, b) in _even_splits(0, HALF, wsplit):
                            nc.sync.dma_start(
                                out=out[t * 128:(t + 1) * 128, a:b],
                                in_=st0[:, a:b],
                            )
                if ngroups >= len(GROUPS):
                    assert all(not v for v in pending.values())
                    for (a, b) in _even_splits(HALF, OUT_COLS, wsplit):
                        nc.sync.dma_start(
                            out=out[t * 128:(t + 1) * 128, a:b],
                            in_=st1[:, a - HALF:b - HALF],
                        )
    nc.compile()
    return nc


def prep_inputs(x: np.ndarray, W: np.ndarray):
    """Full inputs -> per-core in_maps with block-parity-packed bf16 layouts."""
    x = np.ascontiguousarray(np.asarray(x, dtype=np.float32))
    W = np.ascontiguousarray(np.asarray(W, dtype=np.float32))
    wg = W.transpose(1, 0, 2).reshape(D, OUT_COLS)
    w_top = np.zeros((D, W_COLS), np.float32)
    w_bot = np.zeros((D, W_COLS), np.float32)
    for (c0, c1, par, boff) in BLOCKS:
        dst = w_top if par == 0 else w_bot
        dst[:, boff:boff + (c1 - c0)] = wg[:, c0:c1]
    w_p = np.ascontiguousarray(
        np.concatenate([w_top, w_bot], axis=0).astype(ml_dtypes.bfloat16)
    )
    in_maps = []
    for c in range(NCORES):
        xc = x[:, c * BC:(c + 1) * BC, :]                       # [26, 512, 64]
        xn_p = np.ascontiguousarray(
            xc.reshape(F, NT, 128, D).transpose(2, 1, 0, 3)
            .reshape(128, NT * XNW).astype(ml_dtypes.bfloat16)
        )
        xtd = (xc.reshape(F, NT, 128, D).transpose(3, 1, 0, 2)
               .reshape(D, NT * XTW))
        xt_p = np.ascontiguousarray(
            np.concatenate([xtd, xtd], axis=0).astype(ml_dtypes.bfloat16)
        )
        in_maps.append({"xn": xn_p, "xt": xt_p, "w": w_p})
    return in_maps


_CACHED_NC = None


def kernel(x: np.ndarray, W: np.ndarray) -> np.ndarray:
    global _CACHED_NC
    from concourse.bass_utils import run_bass_kernel_spmd

    if _CACHED_NC is None:
        _CACHED_NC = build_bass()
    in_maps = prep_inputs(x, W)
    res = run_bass_kernel_spmd(_CACHED_NC, in_maps, list(range(NCORES)))
    shards = [
        np.asarray(res.results[c]["out"]).astype(np.float32) for c in range(NCORES)
    ]
    return np.concatenate(shards, axis=0)


# revision 34
# speedup vs baseline: 1.3242x; 1.2303x over previous
"""BiLinearInteraction Trainium2 kernel (8 NeuronCores, data-parallel over batch).

Reference computation (per pair p=(i,j) of F=26 fields, P=325 pairs):
    out[b, p*64:(p+1)*64] = (x[i, b, :] @ W[p]) * x[j, b, :]
Full shapes: x [26, 4096, 64] f32, W [325, 64, 64] f32 -> out [4096, 20800] f32.

Strategy (v2)
- Shard batch 4096 -> 8 x 512 (4 batch tiles of 128 rows per core), replicate W.
- Parity-packed operands: even fields' matmul data on SBUF partitions 0-63,
  odd fields' on 64-127 (PE row groups run concurrently via tile_position),
  with NO duplication -> 6.1 MB/core of input HBM traffic (was 10.4).
- All input loads ride the ACT HWDGE ring, output writes the SP HWDGE ring;
  GPSIMD does no DMA descriptor work and is free to run elementwise muls.
- Per batch tile, matmuls accumulate into [128, <=2048] PSUM group tiles
  (4 banks, 2 bufs = whole PSUM). Groups are classed V/A/G:
    V: DVE multiplies straight out of PSUM (fp32, 1x) into the bf16 stage.
    A: one big ACT copy drains the group PSUM -> stage (bf16), then DVE
       multiplies in place at 2x (all-bf16 packed mode).
    G: same drain, but GPSIMD does the in-place mul.
  This spreads the 10.65M-elem/core evacuate+multiply load across all three
  engines (~60us each) instead of DVE+ACT only (~95us DVE in v1).
- Output staged per half batch-tile and written as two ~2.5MB DMAs per tile
  (bf16; host upcasts) -> ~385 GB/s ring efficiency vs ~340 at 0.8MB chunks.
"""

import os
import sys

sys.path.insert(0, "/opt/trn_rl_repo")

from itertools import combinations

import ml_dtypes
import numpy as np

import concourse.bass as bass
import concourse.mybir as mybir
from concourse import bacc
from concourse.tile import TileContext

F, D, B = 26, 64, 4096
NCORES = 8
BC = B // NCORES          # 512 batch rows per core
NT = BC // 128            # 4 batch tiles of 128 rows
PAIRS = list(combinations(range(F), 2))
P = len(PAIRS)            # 325
OUT_COLS = P * D          # 20800

N_PAIRS = [F - 1 - i for i in range(F - 1)]           # pairs with left field i
P_START = [sum(N_PAIRS[:i]) for i in range(F - 1)]    # first pair index of field i
FIELD_START = [P_START[i] * D for i in range(F - 1)]  # output col where field i begins
FIELD_END = [FIELD_START[i] + N_PAIRS[i] * D for i in range(F - 1)]

XTW = F * 128             # xt cols per batch tile (all fields, both halves) = 3328
XNW = F * D               # xn cols per batch tile                           = 1664

# PSUM group grid per batch tile: (c0, c1, class).  <=2048 f32 cols/group
# (4 banks; 2 pool bufs = whole PSUM).
#   V: DVE mul straight from PSUM (fp32, 1x) -- no ACT involvement.
#   A: ACT drain (f32 PSUM -> bf16 stage) + DVE in-place mul at 2x.
# GPSIMD is NOT used for muls: measured on HW, GpSimd tensor_tensor and DVE
# tensor_tensor serialize on the shared SBUF port pair (the blocked op stalls
# for the other's full duration), so GPSIMD adds no elementwise throughput.
# V groups are interleaved between A groups so DVE and ACT stay concurrently
# busy through the tile instead of alternating in bulk phases.  ~21% of cols
# are V: balances DVE (1x V muls + 2x A muls) against ACT (1x drains).
# Halves are field-aligned (field 7 starts at 9856) so muls never span them.
GROUPS = [
    (0, 2048, 'A'), (2048, 4096, 'V'), (4096, 6144, 'A'),
    (6144, 8192, 'A'), (8192, 9856, 'V'),
    (9856, 11904, 'A'), (11904, 13952, 'A'), (13952, 16000, 'A'),
    (16000, 18048, 'A'), (18048, 20096, 'A'), (20096, 20800, 'V'),
]
HALF = 9856               # st0 covers cols [0, 9856), st1 [9856, 20800)

# Contiguous class runs (mul granularity: field pieces within a run).
RUNS = []
for (_g0, _g1, _cls) in GROUPS:
    if RUNS and RUNS[-1][2] == _cls and RUNS[-1][1] == _g0:
        RUNS[-1] = (RUNS[-1][0], _g1, _cls)
    else:
        RUNS.append((_g0, _g1, _cls))

# PSUM 512-col blocks (bank-aligned within each group's psum tile).  The PE
# row group of every matmul is the BLOCK index parity, not the field parity:
# two concurrent (different-row-group) matmuls writing the same PSUM bank is
# a fatal HW collision, so same-bank pieces must share a row group, while
# alternating adjacent blocks keeps dual-row-group concurrency.
# W is packed by block parity: top half (partitions 0-63) holds even blocks'
# columns, bottom half odd blocks'.  xt carries all fields in BOTH halves.
BLOCKS = []               # (c0, c1, parity, w_off)
_tops = _bots = 0
_idx = 0
_V_END_OFF = 0
for (_g0, _g1, _cls) in GROUPS:
    _c = _g0
    while _c < _g1:
        _c1 = min(_c + 512, _g1)
        _par = _idx % 2
        if _par == 0:
            _off = _tops
            _tops += _c1 - _c
        else:
            _off = _bots
            _bots += _c1 - _c
        BLOCKS.append((_c, _c1, _par, _off))
        _idx += 1
        _c = _c1
    if _g1 == 6144:
        _V_END_OFF = max(_tops, _bots)
W_COLS = max(_tops, _bots)
W_SPLIT = -(-_V_END_OFF // 512) * 512     # block-aligned in both halves


def _block_of(col):
    for b in BLOCKS:
        if b[0] <= col < b[1]:
            return b
    raise ValueError(col)

F32 = mybir.dt.float32
BF16 = mybir.dt.bfloat16


def _even_splits(c0, c1, n):
    step = -(-((c1 - c0) // n) // 64) * 64
    step = max(step, 64)
    out = []
    c = c0
    while c < c1:
        out.append((c, min(c + step, c1)))
        c += step
    return out


def _field_of(col):
    for i in range(F - 1):
        if FIELD_START[i] <= col < FIELD_END[i]:
            return i
    raise ValueError(col)


def _pieces(c0, c1, extra=()):
    """Split [c0, c1) at field starts and any extra boundaries.
    Returns list of (p0, p1, field)."""
    bounds = {c0, c1}
    bounds.update(s for s in FIELD_START if c0 < s < c1)
    bounds.update(e for e in extra if c0 < e < c1)
    bs = sorted(bounds)
    return [(a, b, _field_of(a)) for a, b in zip(bs, bs[1:])]


def _mm_pieces(g0, g1):
    """Matmul pieces: additionally split at 512-col PSUM bank boundaries
    (relative to the group base = block boundaries)."""
    extra = set(range(g0 + 512, g1, 512))
    return _pieces(g0, g1, extra)


def build_bass() -> bass.Bass:
    # Bisection flags (default off = full-featured kernel).
    no_gps = os.environ.get("K_NO_GPS", "0") == "1"       # gpsimd muls -> DVE
    no_inplace = os.environ.get("K_NO_INPLACE", "0") == "1"  # muls via cp tile
    swdge_loads = os.environ.get("K_SWDGE_LOADS", "0") == "1"  # loads on gpsimd
    wsplit = int(os.environ.get("K_WRITE_SPLIT", "1"))    # write DMAs per half
    nt_limit = int(os.environ.get("K_NT_LIMIT", str(NT)))  # batch tiles to run
    ngroups = int(os.environ.get("K_NGROUPS", str(len(GROUPS))))
    no_muls = os.environ.get("K_NO_MULS", "0") == "1"
    no_drains = os.environ.get("K_NO_DRAINS", "0") == "1"
    nc = bacc.Bacc()
    xn = nc.declare_dram_parameter("xn", [128, NT * XNW], BF16, isOutput=False)
    xt = nc.declare_dram_parameter("xt", [128, NT * XTW], BF16, isOutput=False)
    w = nc.declare_dram_parameter("w", [128, W_COLS], BF16, isOutput=False)
    out = nc.declare_dram_parameter("out", [BC, OUT_COLS], BF16, isOutput=True)

    with TileContext(nc) as tc:
        with (
            tc.tile_pool(name="consts", bufs=1) as consts,
            tc.tile_pool(name="stage", bufs=4) as stage_pool,
            tc.tile_pool(name="cp", bufs=2) as cp_pool,
            tc.tile_pool(name="psum", bufs=2, space="PSUM") as psum_pool,
        ):
            # Separate tile objects per load DMA keep dependency granularity
            # at the piece level even if subtile range tracking is coarse.
            w_a = consts.tile([128, W_SPLIT], BF16, tag="wa", name="wa")
            w_b = consts.tile([128, W_COLS - W_SPLIT], BF16, tag="wb", name="wb")
            xt0 = consts.tile([128, XTW], BF16, tag="xt0", name="xt0")
            xtr = consts.tile([128, (NT - 1) * XTW], BF16, tag="xtr", name="xtr")
            xn0 = consts.tile([128, XNW], BF16, tag="xn0", name="xn0")
            xnr = consts.tile([128, (NT - 1) * XNW], BF16, tag="xnr", name="xnr")

            # Input loads on the ACT HWDGE ring, ordered so the first batch
            # tile's operands land first (first matmul ~5us in).
            ldeng = nc.gpsimd if swdge_loads else nc.scalar
            ldeng.dma_start(out=xt0[:], in_=xt[:, 0:XTW])
            ldeng.dma_start(out=w_a[:], in_=w[:, 0:W_SPLIT])
            ldeng.dma_start(out=xn0[:], in_=xn[:, 0:XNW])
            ldeng.dma_start(out=w_b[:], in_=w[:, W_SPLIT:W_COLS])
            ldeng.dma_start(out=xtr[:], in_=xt[:, XTW:NT * XTW])
            ldeng.dma_start(out=xnr[:], in_=xn[:, XNW:NT * XNW])

            def xt_slice(t, i, r0):
                c = i * 128
                if t == 0:
                    return xt0[r0:r0 + D, c:c + 128]
                c += (t - 1) * XTW
                return xtr[r0:r0 + D, c:c + 128]

            def w_slice(c0, c1):
                b0, b1, par, boff = _block_of(c0)
                assert c1 <= b1, (c0, c1, b0, b1)
                r0 = par * D
                wc = boff + (c0 - b0)
                n = c1 - c0
                if wc < W_SPLIT:
                    assert wc + n <= W_SPLIT, (c0, c1)
                    return r0, w_a[r0:r0 + D, wc:wc + n]
                return r0, w_b[r0:r0 + D, wc - W_SPLIT:wc - W_SPLIT + n]

            def xn_slice(t, i, c0, c1):
                c = (i + 1) * D + (c0 - FIELD_START[i])
                if t > 0:
                    c += (t - 1) * XNW
                src = xn0 if t == 0 else xnr
                return src[:, c:c + (c1 - c0)]

            for t in range(nt_limit):
                st0 = stage_pool.tile([128, HALF], BF16, tag="stage",
                                      name=f"st{t}a")
                st1 = stage_pool.tile([128, OUT_COLS - HALF], BF16, tag="stage",
                                      name=f"st{t}b")

                def st_slice(c0, c1):
                    if c0 >= HALF:
                        return st1[:, c0 - HALF:c1 - HALF]
                    assert c1 <= HALF
                    return st0[:, c0:c1]

                if no_muls:
                    nc.vector.memset(st0[:], 0.0)
                    nc.vector.memset(st1[:], 0.0)

                # A/G-run mul pieces not yet emitted, per run index.
                pending = {}
                cp_tiles = {}
                for ri, (r0_, r1_, rcls) in enumerate(RUNS):
                    if rcls != 'V':
                        pending[ri] = _pieces(r0_, r1_)
                        if no_inplace:
                            cp_tiles[ri] = cp_pool.tile(
                                [128, r1_ - r0_], BF16, tag="cp",
                                name=f"cp{t}_{ri}")

                gskip = int(os.environ.get("K_GSKIP", "0"))
                mm_filter = os.environ.get("K_MM_FILTER")
                if mm_filter is not None:
                    mm_filter = {int(v) for v in mm_filter.split(",")}
                mm_idx = 0
                for (g0, g1, gcls) in GROUPS[gskip:ngroups]:
                    ps = psum_pool.tile([128, g1 - g0], F32, tag="ps",
                                        name=f"ps{t}_{g0}")
                    for (c0, c1, i) in _mm_pieces(g0, g1):
                        mm_idx += 1
                        if mm_filter is not None and (mm_idx - 1) not in mm_filter:
                            continue
                        r0, rhs = w_slice(c0, c1)
                        nc.tensor.matmul(
                            ps[:, c0 - g0:c1 - g0],
                            xt_slice(t, i, r0),
                            rhs,
                            start=True, stop=True,
                        )
                    if gcls == 'V':
                        for (c0, c1, i) in _pieces(g0, g1):
                            if no_muls:
                                break
                            nc.vector.tensor_mul(
                                st_slice(c0, c1),
                                ps[:, c0 - g0:c1 - g0],
                                xn_slice(t, i, c0, c1),
                            )
                    else:
                        ri = next(k for k, (a, b, cl) in enumerate(RUNS)
                                  if a <= g0 < b)
                        run0 = RUNS[ri][0]
                        # One big ACT drain (f32 PSUM -> bf16, cast).
                        if no_inplace:
                            drain_dst = cp_tiles[ri][:, g0 - run0:g1 - run0]
                        else:
                            drain_dst = st_slice(g0, g1)
                        if not no_drains:
                            nc.scalar.copy(out=drain_dst, in_=ps[:])
                        # Emit muls for run pieces fully drained now.
                        eng = nc.vector if (gcls == 'A' or no_gps) else nc.gpsimd
                        done = [pc for pc in pending[ri] if pc[1] <= g1]
                        for (c0, c1, i) in done:
                            pending[ri].remove((c0, c1, i))
                            if no_muls:
                                continue
                            if no_inplace:
                                msrc = cp_tiles[ri][:, c0 - run0:c1 - run0]
                            else:
                                msrc = st_slice(c0, c1)
                            eng.tensor_mul(
                                st_slice(c0, c1),
                                msrc,
                                xn_slice(t, i, c0, c1),
                            )
                    if g1 == HALF:
                        for (a, b) in _even_splits(0, HALF, wsplit):
                            nc.sync.dma_start(
                                out=out[t * 128:(t + 1) * 128, a:b],
                                in_=st0[:, a:b],
                            )
                if ngroups >= len(GROUPS):
                    assert all(not v for v in pending.values())
                    for (a, b) in _even_splits(HALF, OUT_COLS, wsplit):
                        nc.sync.dma_start(
                            out=out[t * 128:(t + 1) * 128, a:b],
                            in_=st1[:, a - HALF:b - HALF],
                        )
    nc.compile()
    return nc


def prep_inputs(x: np.ndarray, W: np.ndarray):
    """Full inputs -> per-core in_maps with block-parity-packed bf16 layouts."""
    x = np.ascontiguousarray(np.asarray(x, dtype=np.float32))
    W = np.ascontiguousarray(np.asarray(W, dtype=np.float32))
    wg = W.transpose(1, 0, 2).reshape(D, OUT_COLS)
    w_top = np.zeros((D, W_COLS), np.float32)
    w_bot = np.zeros((D, W_COLS), np.float32)
    for (c0, c1, par, boff) in BLOCKS:
        dst = w_top if par == 0 else w_bot
        dst[:, boff:boff + (c1 - c0)] = wg[:, c0:c1]
    w_p = np.ascontiguousarray(
        np.concatenate([w_top, w_bot], axis=0).astype(ml_dtypes.bfloat16)
    )
    in_maps = []
    for c in range(NCORES):
        xc = x[:, c * BC:(c + 1) * BC, :]                       # [26, 512, 64]
        xn_p = np.ascontiguousarray(
            xc.reshape(F, NT, 128, D).transpose(2, 1, 0, 3)
            .reshape(128, NT * XNW).astype(ml_dtypes.bfloat16)
        )
        xtd = (xc.reshape(F, NT, 128, D).transpose(3, 1, 0, 2)
               .reshape(D, NT * XTW))
        xt_p = np.ascontiguousarray(
            np.concatenate([xtd, xtd], axis=0).astype(ml_dtypes.bfloat16)
        )
        in_maps.append({"xn": xn_p, "xt": xt_p, "w": w_p})
    return in_maps


_CACHED_NC = None


def kernel(x: np.ndarray, W: np.ndarray) -> np.ndarray:
    global _CACHED_NC
    from concourse.bass_utils import run_bass_kernel_spmd

    if _CACHED_NC is None:
        _CACHED_NC = build_bass()
    in_maps = prep_inputs(x, W)
    res = run_bass_kernel_spmd(_CACHED_NC, in_maps, list(range(NCORES)))
    shards = [
        np.asarray(res.results[c]["out"]).astype(np.float32) for c in range(NCORES)
    ]
    return np.concatenate(shards, axis=0)


# revision 37
# speedup vs baseline: 1.3619x; 1.0285x over previous
"""BiLinearInteraction Trainium2 kernel (8 NeuronCores, data-parallel over batch).

Reference computation (per pair p=(i,j) of F=26 fields, P=325 pairs):
    out[b, p*64:(p+1)*64] = (x[i, b, :] @ W[p]) * x[j, b, :]
Full shapes: x [26, 4096, 64] f32, W [325, 64, 64] f32 -> out [4096, 20800] f32.

Strategy (v2)
- Shard batch 4096 -> 8 x 512 (4 batch tiles of 128 rows per core), replicate W.
- Parity-packed operands: even fields' matmul data on SBUF partitions 0-63,
  odd fields' on 64-127 (PE row groups run concurrently via tile_position),
  with NO duplication -> 6.1 MB/core of input HBM traffic (was 10.4).
- All input loads ride the ACT HWDGE ring, output writes the SP HWDGE ring;
  GPSIMD does no DMA descriptor work and is free to run elementwise muls.
- Per batch tile, matmuls accumulate into [128, <=2048] PSUM group tiles
  (4 banks, 2 bufs = whole PSUM). Groups are classed V/A/G:
    V: DVE multiplies straight out of PSUM (fp32, 1x) into the bf16 stage.
    A: one big ACT copy drains the group PSUM -> stage (bf16), then DVE
       multiplies in place at 2x (all-bf16 packed mode).
    G: same drain, but GPSIMD does the in-place mul.
  This spreads the 10.65M-elem/core evacuate+multiply load across all three
  engines (~60us each) instead of DVE+ACT only (~95us DVE in v1).
- Output staged per half batch-tile and written as two ~2.5MB DMAs per tile
  (bf16; host upcasts) -> ~385 GB/s ring efficiency vs ~340 at 0.8MB chunks.
"""

import os
import sys

sys.path.insert(0, "/opt/trn_rl_repo")

from itertools import combinations

import ml_dtypes
import numpy as np

import concourse.bass as bass
import concourse.mybir as mybir
from concourse import bacc
from concourse.tile import TileContext

F, D, B = 26, 64, 4096
NCORES = 8
BC = B // NCORES          # 512 batch rows per core
NT = BC // 128            # 4 batch tiles of 128 rows
PAIRS = list(combinations(range(F), 2))
P = len(PAIRS)            # 325
OUT_COLS = P * D          # 20800

N_PAIRS = [F - 1 - i for i in range(F - 1)]           # pairs with left field i
P_START = [sum(N_PAIRS[:i]) for i in range(F - 1)]    # first pair index of field i
FIELD_START = [P_START[i] * D for i in range(F - 1)]  # output col where field i begins
FIELD_END = [FIELD_START[i] + N_PAIRS[i] * D for i in range(F - 1)]

XTW = F * 128             # xt cols per batch tile (all fields, both halves) = 3328
XNW = F * D               # xn cols per batch tile                           = 1664

# PSUM group grid per batch tile: (c0, c1, class).  <=2048 f32 cols/group
# (4 banks; 2 pool bufs = whole PSUM).
#   V: DVE mul straight from PSUM (fp32, 1x) -- no ACT involvement.
#   A: ACT drain (f32 PSUM -> bf16 stage) + DVE in-place mul at 2x.
# GPSIMD is NOT used for muls: measured on HW, GpSimd tensor_tensor and DVE
# tensor_tensor serialize on the shared SBUF port pair (the blocked op stalls
# for the other's full duration), so GPSIMD adds no elementwise throughput.
# V groups are interleaved between A groups so DVE and ACT stay concurrently
# busy through the tile instead of alternating in bulk phases.  ~21% of cols
# are V: balances DVE (1x V muls + 2x A muls) against ACT (1x drains).
# Halves are field-aligned (field 7 starts at 9856) so muls never span them.
GROUPS = [
    (0, 2048, 'A'), (2048, 4096, 'V'), (4096, 6144, 'A'),
    (6144, 8192, 'A'), (8192, 9856, 'V'),
    (9856, 11904, 'A'), (11904, 13952, 'A'), (13952, 16000, 'A'),
    (16000, 18048, 'A'), (18048, 20096, 'A'), (20096, 20800, 'V'),
]
HALF = 9856               # st0 covers cols [0, 9856), st1 [9856, 20800)

# Contiguous class runs (mul granularity: field pieces within a run).
RUNS = []
for (_g0, _g1, _cls) in GROUPS:
    if RUNS and RUNS[-1][2] == _cls and RUNS[-1][1] == _g0:
        RUNS[-1] = (RUNS[-1][0], _g1, _cls)
    else:
        RUNS.append((_g0, _g1, _cls))

# PSUM 512-col blocks (bank-aligned within each group's psum tile).  The PE
# row group of every matmul is the BLOCK index parity, not the field parity:
# two concurrent (different-row-group) matmuls writing the same PSUM bank is
# a fatal HW collision, so same-bank pieces must share a row group, while
# alternating adjacent blocks keeps dual-row-group concurrency.
# W is packed by block parity: top half (partitions 0-63) holds even blocks'
# columns, bottom half odd blocks'.  xt carries all fields in BOTH halves.
BLOCKS = []               # (c0, c1, parity, w_off)
_tops = _bots = 0
_idx = 0
_V_END_OFF = 0
for (_g0, _g1, _cls) in GROUPS:
    _c = _g0
    while _c < _g1:
        _c1 = min(_c + 512, _g1)
        _par = _idx % 2
        if _par == 0:
            _off = _tops
            _tops += _c1 - _c
        else:
            _off = _bots
            _bots += _c1 - _c
        BLOCKS.append((_c, _c1, _par, _off))
        _idx += 1
        _c = _c1
    if _g1 == 6144:
        _V_END_OFF = max(_tops, _bots)
W_COLS = max(_tops, _bots)
W_SPLIT = -(-_V_END_OFF // 512) * 512     # block-aligned in both halves


def _block_of(col):
    for b in BLOCKS:
        if b[0] <= col < b[1]:
            return b
    raise ValueError(col)

F32 = mybir.dt.float32
BF16 = mybir.dt.bfloat16


def _even_splits(c0, c1, n):
    step = -(-((c1 - c0) // n) // 64) * 64
    step = max(step, 64)
    out = []
    c = c0
    while c < c1:
        out.append((c, min(c + step, c1)))
        c += step
    return out


def _field_of(col):
    for i in range(F - 1):
        if FIELD_START[i] <= col < FIELD_END[i]:
            return i
    raise ValueError(col)


def _pieces(c0, c1, extra=()):
    """Split [c0, c1) at field starts and any extra boundaries.
    Returns list of (p0, p1, field)."""
    bounds = {c0, c1}
    bounds.update(s for s in FIELD_START if c0 < s < c1)
    bounds.update(e for e in extra if c0 < e < c1)
    bs = sorted(bounds)
    return [(a, b, _field_of(a)) for a, b in zip(bs, bs[1:])]


def _mm_pieces(g0, g1):
    """Matmul pieces: additionally split at 512-col PSUM bank boundaries
    (relative to the group base = block boundaries)."""
    extra = set(range(g0 + 512, g1, 512))
    return _pieces(g0, g1, extra)


def build_bass() -> bass.Bass:
    # Bisection flags (default off = full-featured kernel).
    no_gps = os.environ.get("K_NO_GPS", "0") == "1"       # gpsimd muls -> DVE
    no_inplace = os.environ.get("K_NO_INPLACE", "0") == "1"  # muls via cp tile
    swdge_loads = os.environ.get("K_SWDGE_LOADS", "0") == "1"  # loads on gpsimd
    wsplit = int(os.environ.get("K_WRITE_SPLIT", "2"))    # write DMAs per half
    nt_limit = int(os.environ.get("K_NT_LIMIT", str(NT)))  # batch tiles to run
    ngroups = int(os.environ.get("K_NGROUPS", str(len(GROUPS))))
    no_muls = os.environ.get("K_NO_MULS", "0") == "1"
    no_drains = os.environ.get("K_NO_DRAINS", "0") == "1"
    nc = bacc.Bacc()
    xn = nc.declare_dram_parameter("xn", [128, NT * XNW], BF16, isOutput=False)
    xt = nc.declare_dram_parameter("xt", [128, NT * XTW], BF16, isOutput=False)
    w = nc.declare_dram_parameter("w", [128, W_COLS], BF16, isOutput=False)
    out = nc.declare_dram_parameter("out", [BC, OUT_COLS], BF16, isOutput=True)

    with TileContext(nc) as tc:
        with (
            tc.tile_pool(name="consts", bufs=1) as consts,
            tc.tile_pool(name="stage", bufs=4) as stage_pool,
            tc.tile_pool(name="cp", bufs=2) as cp_pool,
            tc.tile_pool(name="psum", bufs=2, space="PSUM") as psum_pool,
        ):
            # Separate tile objects per load DMA keep dependency granularity
            # at the piece level.  First-tile operands are split fine so the
            # first matmul can start ~2.5us in; loads ride SWDGE (gpsimd is
            # otherwise idle) so the ACT queue is free for drains.
            HW1 = W_SPLIT // 2
            w_a1 = consts.tile([128, HW1], BF16, tag="wa1", name="wa1")
            w_a2 = consts.tile([128, W_SPLIT - HW1], BF16, tag="wa2", name="wa2")
            w_b = consts.tile([128, W_COLS - W_SPLIT], BF16, tag="wb", name="wb")
            HXT = XTW // 2
            xt0a = consts.tile([128, HXT], BF16, tag="xt0a", name="xt0a")
            xt0b = consts.tile([128, XTW - HXT], BF16, tag="xt0b", name="xt0b")
            xtr = consts.tile([128, (NT - 1) * XTW], BF16, tag="xtr", name="xtr")
            xn0 = consts.tile([128, XNW], BF16, tag="xn0", name="xn0")
            xnr = consts.tile([128, (NT - 1) * XNW], BF16, tag="xnr", name="xnr")

            ldeng = nc.gpsimd if not swdge_loads else nc.scalar
            ldeng.dma_start(out=xt0a[:], in_=xt[:, 0:HXT])
            ldeng.dma_start(out=w_a1[:], in_=w[:, 0:HW1])
            ldeng.dma_start(out=xn0[:], in_=xn[:, 0:XNW])
            ldeng.dma_start(out=w_a2[:], in_=w[:, HW1:W_SPLIT])
            ldeng.dma_start(out=xt0b[:], in_=xt[:, HXT:XTW])
            ldeng.dma_start(out=w_b[:], in_=w[:, W_SPLIT:W_COLS])
            ldeng.dma_start(out=xtr[:], in_=xt[:, XTW:NT * XTW])
            ldeng.dma_start(out=xnr[:], in_=xn[:, XNW:NT * XNW])

            def xt_slice(t, i, r0):
                c = i * 128
                if t == 0:
                    if c + 128 <= HXT:
                        return xt0a[r0:r0 + D, c:c + 128]
                    return xt0b[r0:r0 + D, c - HXT:c - HXT + 128]
                c += (t - 1) * XTW
                return xtr[r0:r0 + D, c:c + 128]

            def w_slice(c0, c1):
                b0, b1, par, boff = _block_of(c0)
                assert c1 <= b1, (c0, c1, b0, b1)
                r0 = par * D
                wc = boff + (c0 - b0)
                n = c1 - c0
                if wc + n <= HW1:
                    return r0, w_a1[r0:r0 + D, wc:wc + n]
                if wc + n <= W_SPLIT:
                    return r0, w_a2[r0:r0 + D, wc - HW1:wc - HW1 + n]
                return r0, w_b[r0:r0 + D, wc - W_SPLIT:wc - W_SPLIT + n]

            def xn_slice(t, i, c0, c1):
                c = (i + 1) * D + (c0 - FIELD_START[i])
                if t > 0:
                    c += (t - 1) * XNW
                src = xn0 if t == 0 else xnr
                return src[:, c:c + (c1 - c0)]

            for t in range(nt_limit):
                st0 = stage_pool.tile([128, HALF], BF16, tag="stage",
                                      name=f"st{t}a")
                st1 = stage_pool.tile([128, OUT_COLS - HALF], BF16, tag="stage",
                                      name=f"st{t}b")

                def st_slice(c0, c1):
                    if c0 >= HALF:
                        return st1[:, c0 - HALF:c1 - HALF]
                    assert c1 <= HALF
                    return st0[:, c0:c1]

                if no_muls:
                    nc.vector.memset(st0[:], 0.0)
                    nc.vector.memset(st1[:], 0.0)

                # A/G-run mul pieces not yet emitted, per run index.
                pending = {}
                cp_tiles = {}
                for ri, (r0_, r1_, rcls) in enumerate(RUNS):
                    if rcls != 'V':
                        pending[ri] = _pieces(r0_, r1_)
                        if no_inplace:
                            cp_tiles[ri] = cp_pool.tile(
                                [128, r1_ - r0_], BF16, tag="cp",
                                name=f"cp{t}_{ri}")

                gskip = int(os.environ.get("K_GSKIP", "0"))
                mm_filter = os.environ.get("K_MM_FILTER")
                if mm_filter is not None:
                    mm_filter = {int(v) for v in mm_filter.split(",")}
                mm_idx = 0
                for (g0, g1, gcls) in GROUPS[gskip:ngroups]:
                    ps = psum_pool.tile([128, g1 - g0], F32, tag="ps",
                                        name=f"ps{t}_{g0}")
                    for (c0, c1, i) in _mm_pieces(g0, g1):
                        mm_idx += 1
                        if mm_filter is not None and (mm_idx - 1) not in mm_filter:
                            continue
                        r0, rhs = w_slice(c0, c1)
                        nc.tensor.matmul(
                            ps[:, c0 - g0:c1 - g0],
                            xt_slice(t, i, r0),
                            rhs,
                            start=True, stop=True,
                        )
                    if gcls == 'V':
                        for (c0, c1, i) in _pieces(g0, g1):
                            if no_muls:
                                break
                            nc.vector.tensor_mul(
                                st_slice(c0, c1),
                                ps[:, c0 - g0:c1 - g0],
                                xn_slice(t, i, c0, c1),
                            )
                    else:
                        ri = next(k for k, (a, b, cl) in enumerate(RUNS)
                                  if a <= g0 < b)
                        run0 = RUNS[ri][0]
                        # One big ACT drain (f32 PSUM -> bf16, cast).
                        if no_inplace:
                            drain_dst = cp_tiles[ri][:, g0 - run0:g1 - run0]
                        else:
                            drain_dst = st_slice(g0, g1)
                        if not no_drains:
                            nc.scalar.copy(out=drain_dst, in_=ps[:])
                        # Emit muls for run pieces fully drained now.
                        eng = nc.vector if (gcls == 'A' or no_gps) else nc.gpsimd
                        done = [pc for pc in pending[ri] if pc[1] <= g1]
                        for (c0, c1, i) in done:
                            pending[ri].remove((c0, c1, i))
                            if no_muls:
                                continue
                            if no_inplace:
                                msrc = cp_tiles[ri][:, c0 - run0:c1 - run0]
                            else:
                                msrc = st_slice(c0, c1)
                            eng.tensor_mul(
                                st_slice(c0, c1),
                                msrc,
                                xn_slice(t, i, c0, c1),
                            )
                    if g1 == HALF:
                        for (a, b) in _even_splits(0, HALF, wsplit):
                            nc.sync.dma_start(
                                out=out[t * 128:(t + 1) * 128, a:b],
                                in_=st0[:, a:b],
                            )
                if ngroups >= len(GROUPS):
                    assert all(not v for v in pending.values())
                    for (a, b) in _even_splits(HALF, OUT_COLS, wsplit):
                        nc.sync.dma_start(
                            out=out[t * 128:(t + 1) * 128, a:b],
                            in_=st1[:, a - HALF:b - HALF],
                        )
    nc.compile()
    return nc


def prep_inputs(x: np.ndarray, W: np.ndarray):
    """Full inputs -> per-core in_maps with block-parity-packed bf16 layouts."""
    x = np.ascontiguousarray(np.asarray(x, dtype=np.float32))
    W = np.ascontiguousarray(np.asarray(W, dtype=np.float32))
    wg = W.transpose(1, 0, 2).reshape(D, OUT_COLS)
    w_top = np.zeros((D, W_COLS), np.float32)
    w_bot = np.zeros((D, W_COLS), np.float32)
    for (c0, c1, par, boff) in BLOCKS:
        dst = w_top if par == 0 else w_bot
        dst[:, boff:boff + (c1 - c0)] = wg[:, c0:c1]
    w_p = np.ascontiguousarray(
        np.concatenate([w_top, w_bot], axis=0).astype(ml_dtypes.bfloat16)
    )
    in_maps = []
    for c in range(NCORES):
        xc = x[:, c * BC:(c + 1) * BC, :]                       # [26, 512, 64]
        xn_p = np.ascontiguousarray(
            xc.reshape(F, NT, 128, D).transpose(2, 1, 0, 3)
            .reshape(128, NT * XNW).astype(ml_dtypes.bfloat16)
        )
        xtd = (xc.reshape(F, NT, 128, D).transpose(3, 1, 0, 2)
               .reshape(D, NT * XTW))
        xt_p = np.ascontiguousarray(
            np.concatenate([xtd, xtd], axis=0).astype(ml_dtypes.bfloat16)
        )
        in_maps.append({"xn": xn_p, "xt": xt_p, "w": w_p})
    return in_maps


_CACHED_NC = None


def kernel(x: np.ndarray, W: np.ndarray) -> np.ndarray:
    global _CACHED_NC
    from concourse.bass_utils import run_bass_kernel_spmd

    if _CACHED_NC is None:
        _CACHED_NC = build_bass()
    in_maps = prep_inputs(x, W)
    res = run_bass_kernel_spmd(_CACHED_NC, in_maps, list(range(NCORES)))
    shards = [
        np.asarray(res.results[c]["out"]).astype(np.float32) for c in range(NCORES)
    ]
    return np.concatenate(shards, axis=0)


# revision 38
# speedup vs baseline: 1.3783x; 1.0120x over previous
"""BiLinearInteraction Trainium2 kernel (8 NeuronCores, data-parallel over batch).

Reference computation (per pair p=(i,j) of F=26 fields, P=325 pairs):
    out[b, p*64:(p+1)*64] = (x[i, b, :] @ W[p]) * x[j, b, :]
Full shapes: x [26, 4096, 64] f32, W [325, 64, 64] f32 -> out [4096, 20800] f32.

Strategy (v2)
- Shard batch 4096 -> 8 x 512 (4 batch tiles of 128 rows per core), replicate W.
- Parity-packed operands: even fields' matmul data on SBUF partitions 0-63,
  odd fields' on 64-127 (PE row groups run concurrently via tile_position),
  with NO duplication -> 6.1 MB/core of input HBM traffic (was 10.4).
- All input loads ride the ACT HWDGE ring, output writes the SP HWDGE ring;
  GPSIMD does no DMA descriptor work and is free to run elementwise muls.
- Per batch tile, matmuls accumulate into [128, <=2048] PSUM group tiles
  (4 banks, 2 bufs = whole PSUM). Groups are classed V/A/G:
    V: DVE multiplies straight out of PSUM (fp32, 1x) into the bf16 stage.
    A: one big ACT copy drains the group PSUM -> stage (bf16), then DVE
       multiplies in place at 2x (all-bf16 packed mode).
    G: same drain, but GPSIMD does the in-place mul.
  This spreads the 10.65M-elem/core evacuate+multiply load across all three
  engines (~60us each) instead of DVE+ACT only (~95us DVE in v1).
- Output staged per half batch-tile and written as two ~2.5MB DMAs per tile
  (bf16; host upcasts) -> ~385 GB/s ring efficiency vs ~340 at 0.8MB chunks.
"""

import os
import sys

sys.path.insert(0, "/opt/trn_rl_repo")

from itertools import combinations

import ml_dtypes
import numpy as np

import concourse.bass as bass
import concourse.mybir as mybir
from concourse import bacc
from concourse.tile import TileContext

F, D, B = 26, 64, 4096
NCORES = 8
BC = B // NCORES          # 512 batch rows per core
NT = BC // 128            # 4 batch tiles of 128 rows
PAIRS = list(combinations(range(F), 2))
P = len(PAIRS)            # 325
OUT_COLS = P * D          # 20800

N_PAIRS = [F - 1 - i for i in range(F - 1)]           # pairs with left field i
P_START = [sum(N_PAIRS[:i]) for i in range(F - 1)]    # first pair index of field i
FIELD_START = [P_START[i] * D for i in range(F - 1)]  # output col where field i begins
FIELD_END = [FIELD_START[i] + N_PAIRS[i] * D for i in range(F - 1)]

XTW = F * 128             # xt cols per batch tile (all fields, both halves) = 3328
XNW = F * D               # xn cols per batch tile                           = 1664

# PSUM group grid per batch tile: (c0, c1, class).  <=2048 f32 cols/group
# (4 banks; 2 pool bufs = whole PSUM).
#   V: DVE mul straight from PSUM (fp32, 1x) -- no ACT involvement.
#   A: ACT drain (f32 PSUM -> bf16 stage) + DVE in-place mul at 2x.
# GPSIMD is NOT used for muls: measured on HW, GpSimd tensor_tensor and DVE
# tensor_tensor serialize on the shared SBUF port pair (the blocked op stalls
# for the other's full duration), so GPSIMD adds no elementwise throughput.
# V groups are interleaved between A groups so DVE and ACT stay concurrently
# busy through the tile instead of alternating in bulk phases.  ~21% of cols
# are V: balances DVE (1x V muls + 2x A muls) against ACT (1x drains).
# Halves are field-aligned (field 7 starts at 9856) so muls never span them.
GROUPS = [
    (0, 2048, 'A'), (2048, 4096, 'V'), (4096, 6144, 'A'),
    (6144, 8192, 'A'), (8192, 9856, 'V'),
    (9856, 11904, 'A'), (11904, 13952, 'A'), (13952, 16000, 'A'),
    (16000, 18048, 'A'), (18048, 20096, 'A'), (20096, 20800, 'V'),
]
HALF = 9856               # st0 covers cols [0, 9856), st1 [9856, 20800)

# Contiguous class runs (mul granularity: field pieces within a run).
RUNS = []
for (_g0, _g1, _cls) in GROUPS:
    if RUNS and RUNS[-1][2] == _cls and RUNS[-1][1] == _g0:
        RUNS[-1] = (RUNS[-1][0], _g1, _cls)
    else:
        RUNS.append((_g0, _g1, _cls))

# PSUM 512-col blocks (bank-aligned within each group's psum tile).  The PE
# row group of every matmul is the BLOCK index parity, not the field parity:
# two concurrent (different-row-group) matmuls writing the same PSUM bank is
# a fatal HW collision, so same-bank pieces must share a row group, while
# alternating adjacent blocks keeps dual-row-group concurrency.
# W is packed by block parity: top half (partitions 0-63) holds even blocks'
# columns, bottom half odd blocks'.  xt carries all fields in BOTH halves.
BLOCKS = []               # (c0, c1, parity, w_off)
_tops = _bots = 0
_idx = 0
_V_END_OFF = 0
for (_g0, _g1, _cls) in GROUPS:
    _c = _g0
    while _c < _g1:
        _c1 = min(_c + 512, _g1)
        _par = _idx % 2
        if _par == 0:
            _off = _tops
            _tops += _c1 - _c
        else:
            _off = _bots
            _bots += _c1 - _c
        BLOCKS.append((_c, _c1, _par, _off))
        _idx += 1
        _c = _c1
    if _g1 == 6144:
        _V_END_OFF = max(_tops, _bots)
W_COLS = max(_tops, _bots)
W_SPLIT = -(-_V_END_OFF // 512) * 512     # block-aligned in both halves


def _block_of(col):
    for b in BLOCKS:
        if b[0] <= col < b[1]:
            return b
    raise ValueError(col)

F32 = mybir.dt.float32
BF16 = mybir.dt.bfloat16


def _even_splits(c0, c1, n):
    step = -(-((c1 - c0) // n) // 64) * 64
    step = max(step, 64)
    out = []
    c = c0
    while c < c1:
        out.append((c, min(c + step, c1)))
        c += step
    return out


def _field_of(col):
    for i in range(F - 1):
        if FIELD_START[i] <= col < FIELD_END[i]:
            return i
    raise ValueError(col)


def _pieces(c0, c1, extra=()):
    """Split [c0, c1) at field starts and any extra boundaries.
    Returns list of (p0, p1, field)."""
    bounds = {c0, c1}
    bounds.update(s for s in FIELD_START if c0 < s < c1)
    bounds.update(e for e in extra if c0 < e < c1)
    bs = sorted(bounds)
    return [(a, b, _field_of(a)) for a, b in zip(bs, bs[1:])]


def _mm_pieces(g0, g1):
    """Matmul pieces: additionally split at 512-col PSUM bank boundaries
    (relative to the group base = block boundaries)."""
    extra = set(range(g0 + 512, g1, 512))
    return _pieces(g0, g1, extra)


def build_bass() -> bass.Bass:
    # Bisection flags (default off = full-featured kernel).
    no_gps = os.environ.get("K_NO_GPS", "0") == "1"       # gpsimd muls -> DVE
    no_inplace = os.environ.get("K_NO_INPLACE", "0") == "1"  # muls via cp tile
    swdge_loads = os.environ.get("K_SWDGE_LOADS", "0") == "1"  # loads on gpsimd
    wsplit = int(os.environ.get("K_WRITE_SPLIT", "2"))    # write DMAs per half
    nt_limit = int(os.environ.get("K_NT_LIMIT", str(NT)))  # batch tiles to run
    ngroups = int(os.environ.get("K_NGROUPS", str(len(GROUPS))))
    no_muls = os.environ.get("K_NO_MULS", "0") == "1"
    no_drains = os.environ.get("K_NO_DRAINS", "0") == "1"
    nc = bacc.Bacc()
    xn = nc.declare_dram_parameter("xn", [128, NT * XNW], BF16, isOutput=False)
    xt = nc.declare_dram_parameter("xt", [128, NT * XTW], BF16, isOutput=False)
    w = nc.declare_dram_parameter("w", [128, W_COLS], BF16, isOutput=False)
    out = nc.declare_dram_parameter("out", [BC, OUT_COLS], BF16, isOutput=True)

    with TileContext(nc) as tc:
        with (
            tc.tile_pool(name="consts", bufs=1) as consts,
            tc.tile_pool(name="stage", bufs=4) as stage_pool,
            tc.tile_pool(name="cp", bufs=2) as cp_pool,
            tc.tile_pool(name="psum", bufs=2, space="PSUM") as psum_pool,
        ):
            # Separate tile objects per load DMA keep dependency granularity
            # at the piece level.  First-tile operands are split fine so the
            # first matmul can start ~2.5us in; loads ride SWDGE (gpsimd is
            # otherwise idle) so the ACT queue is free for drains.
            HW1 = W_SPLIT // 2
            w_a1 = consts.tile([128, HW1], BF16, tag="wa1", name="wa1")
            w_a2 = consts.tile([128, W_SPLIT - HW1], BF16, tag="wa2", name="wa2")
            w_b = consts.tile([128, W_COLS - W_SPLIT], BF16, tag="wb", name="wb")
            HXT = XTW // 2
            xt0a = consts.tile([128, HXT], BF16, tag="xt0a", name="xt0a")
            xt0b = consts.tile([128, XTW - HXT], BF16, tag="xt0b", name="xt0b")
            xtr = consts.tile([128, (NT - 1) * XTW], BF16, tag="xtr", name="xtr")
            xn0 = consts.tile([128, XNW], BF16, tag="xn0", name="xn0")
            xnr = consts.tile([128, (NT - 1) * XNW], BF16, tag="xnr", name="xnr")

            ldeng = nc.sync if not swdge_loads else nc.gpsimd
            ldeng.dma_start(out=xt0a[:], in_=xt[:, 0:HXT])
            ldeng.dma_start(out=w_a1[:], in_=w[:, 0:HW1])
            ldeng.dma_start(out=xn0[:], in_=xn[:, 0:XNW])
            ldeng.dma_start(out=w_a2[:], in_=w[:, HW1:W_SPLIT])
            ldeng.dma_start(out=xt0b[:], in_=xt[:, HXT:XTW])
            ldeng.dma_start(out=w_b[:], in_=w[:, W_SPLIT:W_COLS])
            ldeng.dma_start(out=xtr[:], in_=xt[:, XTW:NT * XTW])
            ldeng.dma_start(out=xnr[:], in_=xn[:, XNW:NT * XNW])

            def xt_slice(t, i, r0):
                c = i * 128
                if t == 0:
                    if c + 128 <= HXT:
                        return xt0a[r0:r0 + D, c:c + 128]
                    return xt0b[r0:r0 + D, c - HXT:c - HXT + 128]
                c += (t - 1) * XTW
                return xtr[r0:r0 + D, c:c + 128]

            def w_slice(c0, c1):
                b0, b1, par, boff = _block_of(c0)
                assert c1 <= b1, (c0, c1, b0, b1)
                r0 = par * D
                wc = boff + (c0 - b0)
                n = c1 - c0
                if wc + n <= HW1:
                    return r0, w_a1[r0:r0 + D, wc:wc + n]
                if wc + n <= W_SPLIT:
                    return r0, w_a2[r0:r0 + D, wc - HW1:wc - HW1 + n]
                return r0, w_b[r0:r0 + D, wc - W_SPLIT:wc - W_SPLIT + n]

            def xn_slice(t, i, c0, c1):
                c = (i + 1) * D + (c0 - FIELD_START[i])
                if t > 0:
                    c += (t - 1) * XNW
                src = xn0 if t == 0 else xnr
                return src[:, c:c + (c1 - c0)]

            for t in range(nt_limit):
                st0 = stage_pool.tile([128, HALF], BF16, tag="stage",
                                      name=f"st{t}a")
                st1 = stage_pool.tile([128, OUT_COLS - HALF], BF16, tag="stage",
                                      name=f"st{t}b")

                def st_slice(c0, c1):
                    if c0 >= HALF:
                        return st1[:, c0 - HALF:c1 - HALF]
                    assert c1 <= HALF
                    return st0[:, c0:c1]

                if no_muls:
                    nc.vector.memset(st0[:], 0.0)
                    nc.vector.memset(st1[:], 0.0)

                # A/G-run mul pieces not yet emitted, per run index.
                pending = {}
                cp_tiles = {}
                for ri, (r0_, r1_, rcls) in enumerate(RUNS):
                    if rcls != 'V':
                        pending[ri] = _pieces(r0_, r1_)
                        if no_inplace:
                            cp_tiles[ri] = cp_pool.tile(
                                [128, r1_ - r0_], BF16, tag="cp",
                                name=f"cp{t}_{ri}")

                gskip = int(os.environ.get("K_GSKIP", "0"))
                mm_filter = os.environ.get("K_MM_FILTER")
                if mm_filter is not None:
                    mm_filter = {int(v) for v in mm_filter.split(",")}
                mm_idx = 0
                for (g0, g1, gcls) in GROUPS[gskip:ngroups]:
                    ps = psum_pool.tile([128, g1 - g0], F32, tag="ps",
                                        name=f"ps{t}_{g0}")
                    for (c0, c1, i) in _mm_pieces(g0, g1):
                        mm_idx += 1
                        if mm_filter is not None and (mm_idx - 1) not in mm_filter:
                            continue
                        r0, rhs = w_slice(c0, c1)
                        nc.tensor.matmul(
                            ps[:, c0 - g0:c1 - g0],
                            xt_slice(t, i, r0),
                            rhs,
                            start=True, stop=True,
                        )
                    if gcls == 'V':
                        for (c0, c1, i) in _pieces(g0, g1):
                            if no_muls:
                                break
                            nc.vector.tensor_mul(
                                st_slice(c0, c1),
                                ps[:, c0 - g0:c1 - g0],
                                xn_slice(t, i, c0, c1),
                            )
                    else:
                        ri = next(k for k, (a, b, cl) in enumerate(RUNS)
                                  if a <= g0 < b)
                        run0 = RUNS[ri][0]
                        # One big ACT drain (f32 PSUM -> bf16, cast).
                        if no_inplace:
                            drain_dst = cp_tiles[ri][:, g0 - run0:g1 - run0]
                        else:
                            drain_dst = st_slice(g0, g1)
                        if not no_drains:
                            nc.scalar.copy(out=drain_dst, in_=ps[:])
                        # Emit muls for run pieces fully drained now.
                        eng = nc.vector if (gcls == 'A' or no_gps) else nc.gpsimd
                        done = [pc for pc in pending[ri] if pc[1] <= g1]
                        for (c0, c1, i) in done:
                            pending[ri].remove((c0, c1, i))
                            if no_muls:
                                continue
                            if no_inplace:
                                msrc = cp_tiles[ri][:, c0 - run0:c1 - run0]
                            else:
                                msrc = st_slice(c0, c1)
                            eng.tensor_mul(
                                st_slice(c0, c1),
                                msrc,
                                xn_slice(t, i, c0, c1),
                            )
                    if g1 == HALF:
                        for (a, b) in _even_splits(0, HALF, wsplit):
                            nc.sync.dma_start(
                                out=out[t * 128:(t + 1) * 128, a:b],
                                in_=st0[:, a:b],
                            )
                if ngroups >= len(GROUPS):
                    assert all(not v for v in pending.values())
                    for (a, b) in _even_splits(HALF, OUT_COLS, wsplit):
                        nc.sync.dma_start(
                            out=out[t * 128:(t + 1) * 128, a:b],
                            in_=st1[:, a - HALF:b - HALF],
                        )
    nc.compile()
    return nc


def prep_inputs(x: np.ndarray, W: np.ndarray):
    """Full inputs -> per-core in_maps with block-parity-packed bf16 layouts."""
    x = np.ascontiguousarray(np.asarray(x, dtype=np.float32))
    W = np.ascontiguousarray(np.asarray(W, dtype=np.float32))
    wg = W.transpose(1, 0, 2).reshape(D, OUT_COLS)
    w_top = np.zeros((D, W_COLS), np.float32)
    w_bot = np.zeros((D, W_COLS), np.float32)
    for (c0, c1, par, boff) in BLOCKS:
        dst = w_top if par == 0 else w_bot
        dst[:, boff:boff + (c1 - c0)] = wg[:, c0:c1]
    w_p = np.ascontiguousarray(
        np.concatenate([w_top, w_bot], axis=0).astype(ml_dtypes.bfloat16)
    )
    in_maps = []
    for c in range(NCORES):
        xc = x[:, c * BC:(c + 1) * BC, :]                       # [26, 512, 64]
        xn_p = np.ascontiguousarray(
            xc.reshape(F, NT, 128, D).transpose(2, 1, 0, 3)
            .reshape(128, NT * XNW).astype(ml_dtypes.bfloat16)
        )
        xtd = (xc.reshape(F, NT, 128, D).transpose(3, 1, 0, 2)
               .reshape(D, NT * XTW))
        xt_p = np.ascontiguousarray(
            np.concatenate([xtd, xtd], axis=0).astype(ml_dtypes.bfloat16)
        )
        in_maps.append({"xn": xn_p, "xt": xt_p, "w": w_p})
    return in_maps


_CACHED_NC = None


def kernel(x: np.ndarray, W: np.ndarray) -> np.ndarray:
    global _CACHED_NC
    from concourse.bass_utils import run_bass_kernel_spmd

    if _CACHED_NC is None:
        _CACHED_NC = build_bass()
    in_maps = prep_inputs(x, W)
    res = run_bass_kernel_spmd(_CACHED_NC, in_maps, list(range(NCORES)))
    shards = [
        np.asarray(res.results[c]["out"]).astype(np.float32) for c in range(NCORES)
    ]
    return np.concatenate(shards, axis=0)
